# revision 2
# baseline (speedup 1.0000x reference)
"""Trainium2 Bass kernel for DeepSeek-style SwiGLU MLP.

    ug   = x @ w_up_gate.T          # [M, 2E]
    h    = silu(ug[:, :E]) * ug[:, E:]
    out  = h @ w_down.T             # [M, K]

Shapes: M=4096 tokens, K=7168 hidden, E=2048 expert dim (fp32 in/out).

Sharding: data-parallel over tokens — each of the 8 cores computes 512
token rows end-to-end with the full weights. No cross-core collectives;
host concatenates the per-core row blocks.

Per-core dataflow (all "transposed" so no on-chip transposes are needed):
  GEMM1: ugT[n, m] = sum_k wugT[k, n] * xT[k, m]
         -> psum tiles [128(n), 512(m)], lhsT = w tile, rhs = xT tile
  SwiGLU: hT[e, m] = silu(ugT[e, m]) * ugT[E+e, m]   (ACT silu + DVE mul)
  GEMM2: outT[hid, m] = sum_e wdT[e, hid] * hT[e, m]
         -> psum tiles [128(hid), 512(m)]
Host packs x / weights into bf16 partition-major layouts; matmuls run in
bf16 with fp32 PSUM accumulation; output is fp32.
"""

import numpy as np
import ml_dtypes
from contextlib import ExitStack

import concourse.bass as bass
import concourse.mybir as mybir
import concourse.tile as tile
from concourse import bacc
from concourse.bass_utils import run_bass_kernel_spmd

M, K, E = 4096, 7168, 2048
N_CORES = 8
P = 128
MS = M // N_CORES        # 512 tokens per core
KT = K // P              # 56 contraction tiles for GEMM1 / output tiles for GEMM2
ET = E // P              # 16 expert tiles
NT2 = 2 * ET             # 32 row tiles of w_up_gate

BF16 = mybir.dt.bfloat16
F32 = mybir.dt.float32

_CACHE: dict = {}


def _build_program():
    nc = bacc.Bacc(
        "TRN2", target_bir_lowering=False, debug=False, num_devices=N_CORES
    )

    xt_d = nc.dram_tensor("xt", [P, KT * MS], BF16, kind="ExternalInput").ap()
    wug_d = nc.dram_tensor("wug", [NT2, P, KT * P], BF16, kind="ExternalInput").ap()
    wd_d = nc.dram_tensor("wd", [KT, P, ET * P], BF16, kind="ExternalInput").ap()
    out_d = nc.dram_tensor("out", [KT, P, MS], F32, kind="ExternalOutput").ap()

    xt_r = xt_d.rearrange("p (kt m) -> p kt m", kt=KT)
    wug_r = wug_d.rearrange("n p (kt w) -> n p kt w", kt=KT)
    wd_r = wd_d.rearrange("h p (et w) -> h p et w", et=ET)

    with tile.TileContext(nc) as tc, ExitStack() as ctx:
        xpool = ctx.enter_context(tc.tile_pool(name="xpool", bufs=1))
        hpool = ctx.enter_context(tc.tile_pool(name="hpool", bufs=1))
        wpool = ctx.enter_context(tc.tile_pool(name="wpool", bufs=2))
        wdpool = ctx.enter_context(tc.tile_pool(name="wdpool", bufs=3))
        spool = ctx.enter_context(tc.tile_pool(name="spool", bufs=2))
        opool = ctx.enter_context(tc.tile_pool(name="opool", bufs=3))
        psum1 = ctx.enter_context(tc.tile_pool(name="psum1", bufs=2, space="PSUM"))
        psum2 = ctx.enter_context(tc.tile_pool(name="psum2", bufs=4, space="PSUM"))

        # xT resident in SBUF for the whole kernel (56 KB/partition).
        # Chunked DMA so early matmuls can start before the full load lands.
        xt_sb = xpool.tile([P, KT, MS], BF16)
        for kt in range(KT):
            nc.sync.dma_start(xt_sb[:, kt, :], xt_r[:, kt, :])

        # hT (SwiGLU output, GEMM2 rhs), 16 KB/partition.
        ht_sb = hpool.tile([P, ET, MS], BF16)

        # ---- Phase 1: up/gate GEMM + SwiGLU, one expert row-tile at a time
        for et in range(ET):
            wg = wpool.tile([P, KT, P], BF16, tag="w")
            nc.sync.dma_start(wg[:], wug_r[et])
            wu = wpool.tile([P, KT, P], BF16, tag="w")
            nc.sync.dma_start(wu[:], wug_r[ET + et])

            ps_g = psum1.tile([P, MS], F32, tag="ps")
            for kt in range(KT):
                nc.tensor.matmul(
                    ps_g[:], wg[:, kt, :], xt_sb[:, kt, :],
                    start=(kt == 0), stop=(kt == KT - 1),
                )
            ps_u = psum1.tile([P, MS], F32, tag="ps")
            for kt in range(KT):
                nc.tensor.matmul(
                    ps_u[:], wu[:, kt, :], xt_sb[:, kt, :],
                    start=(kt == 0), stop=(kt == KT - 1),
                )

            # silu(g)*u = g*sigmoid(g)*u — Sigmoid (not Silu) so CoreSim can
            # check the exact program; each DVE mul reads at most one PSUM AP.
            sig = spool.tile([P, MS], F32, tag="sig")
            nc.scalar.activation(sig[:], ps_g[:], mybir.ActivationFunctionType.Sigmoid)
            gsig = spool.tile([P, MS], F32, tag="gsig")
            nc.vector.tensor_mul(gsig[:], sig[:], ps_g[:])
            nc.vector.tensor_mul(ht_sb[:, et, :], gsig[:], ps_u[:])

        # ---- Phase 2: down GEMM, one hidden row-tile at a time
        for ht in range(KT):
            wdt = wdpool.tile([P, ET, P], BF16)
            nc.sync.dma_start(wdt[:], wd_r[ht])

            ps_o = psum2.tile([P, MS], F32)
            for et in range(ET):
                nc.tensor.matmul(
                    ps_o[:], wdt[:, et, :], ht_sb[:, et, :],
                    start=(et == 0), stop=(et == ET - 1),
                )
            ot = opool.tile([P, MS], F32)
            nc.any.tensor_copy(ot[:], ps_o[:])
            nc.sync.dma_start(out_d[ht], ot[:])

    nc.compile()
    return nc


def get_program():
    if "nc" not in _CACHE:
        _CACHE["nc"] = _build_program()
    return _CACHE["nc"]


def pack_inputs(x, w_up_gate, w_down):
    """Host-side shard + relayout into the kernel's DRAM layouts (bf16)."""
    bf = ml_dtypes.bfloat16
    # xt[c][p, kt*MS + m] = x[c*MS + m, kt*P + p]
    xp = np.ascontiguousarray(
        x.astype(bf).reshape(N_CORES, MS, KT, P).transpose(0, 3, 2, 1)
    ).reshape(N_CORES, P, KT * MS)
    # wug[nt, p, kt*P + n2] = w_up_gate[nt*P + n2, kt*P + p]
    wugp = np.ascontiguousarray(
        w_up_gate.astype(bf).reshape(NT2, P, KT, P).transpose(0, 3, 2, 1)
    ).reshape(NT2, P, KT * P)
    # wd[ht, p, et*P + n2] = w_down[ht*P + n2, et*P + p]
    wdp = np.ascontiguousarray(
        w_down.astype(bf).reshape(KT, P, ET, P).transpose(0, 3, 2, 1)
    ).reshape(KT, P, ET * P)
    return xp, wugp, wdp


def unpack_output(results):
    """results[c]["out"] is [KT, P, MS] fp32 = outT blocks; back to [M, K]."""
    out = np.empty((M, K), np.float32)
    for c in range(N_CORES):
        blk = np.asarray(results[c]["out"], np.float32)  # [KT, P, MS]
        out[c * MS : (c + 1) * MS] = blk.transpose(2, 0, 1).reshape(MS, K)
    return out


def kernel(x, w_up_gate, w_down):
    nc = get_program()
    xp, wugp, wdp = pack_inputs(x, w_up_gate, w_down)
    in_maps = [{"xt": xp[c], "wug": wugp, "wd": wdp} for c in range(N_CORES)]
    res = run_bass_kernel_spmd(nc, in_maps, core_ids=list(range(N_CORES)))
    return unpack_output(res.results)
